# revision 93
# baseline (speedup 1.0000x reference)
"""Linformer attention TRN2 kernel (8 NeuronCores), v4.

Sharding: core c handles batch b = c//2 and head-half hh = c%2
(8 of 16 heads = 512 of 1024 feature columns of Wq/Wk/Wv, and the
matching 512 rows of Wo). Each core computes a partial output
y_part = attn_out_half @ Wo[hh*512:(hh+1)*512, :]; the host sums the
two partials per batch and adds bo.

Phase 1 uses the Linformer algebra refactor:
    kE = (x Wk + bk)^T E = Wk^T (x^T E) + bk (1^T E)
so the big [L,D]x[D,DH] k/v projections collapse into one shared
z = x^T [E|F] ([D, 2K], accumulated over L in 8 PSUM banks, x/EF as
bf16), followed by tiny [D,DH]^T[D,K] projections and rank-1 bias
matmuls (bk x sum(E), sum(F) x bv). This cuts phase-1 PE cycles
~2.2x and nearly halves phase-1 DMA.

Phase 2 is transpose-free: scores are computed directly in
transposed [k, l] layout (sT = kET_h^T @ qT_h). Softmax shift
exploits shift-invariance: one global constant M (midpoint between
the global max and the smallest row max of tile-0/head-0 natural-
layout scores, +12 tilt), computed once on-device, keeps every
exp(s - M) inside f32/bf16 dynamic range, so no per-row max pass is
needed. vF carries 64 replicated ones-columns, so the pv matmul
(out = vF_aug_h^T @ sT) yields Z = sum_k exp replicated in PSUM rows
64:128 for free; the drain stages Z via ACT, takes a fast approx
reciprocal and normalizes on DVE while writing outT (bf16).
y(lt) = outT^T @ Wo (bf16) drains split DVE/ACT and DMAs per half.
Per tile, scores/exp/pv per head are threaded through 2 score PSUM
buffers with q(lt+1) and y(lt-1) interleaved into 8 PE gap slots.

All streamed inputs are host-swizzled partition-major so each DMA
reads multi-KB contiguous runs per partition line.

The per-head exp is split per k-chunk so each pv matmul waits on
only the half of the exp it consumes.

Measured: ~300 us HW exec (vs 485 us v2 baseline), rel_l2 ~1.25e-2.
"""

import sys

sys.path.insert(0, "/opt/trn_rl_repo")

import numpy as np
import ml_dtypes

import concourse.bass as bass
import concourse.mybir as mybir
import concourse.tile as tile
from concourse import bacc
from concourse import bass_utils

B, L, D, H, HD, K = 4, 4096, 1024, 16, 64, 256
DH = 512                      # per-core feature slice (8 heads x 64)
NHL = 8                       # heads per core
SCALE = HD ** -0.5
P = 128
LS1 = 512                     # phase-1 L slice
NS1 = L // LS1                # 8 phase-1 iterations
LT2 = 512                     # phase-2 L tile
NT2 = L // LT2                # 8 phase-2 iterations
F32 = mybir.dt.float32
F32R = mybir.dt.float32r
BF16 = mybir.dt.bfloat16
FP16 = mybir.dt.float16

MMDT = F32R                   # dtype of every matmul-feeding tensor

_CACHE = {}


def build_program():
    nc = bacc.Bacc("TRN2", target_bir_lowering=False, debug=False)

    # pre-swizzled on host so every DMA is a long contiguous run per
    # partition: xt[p, t, c, :] = x^T[c*128+p, t*512:(t+1)*512] etc.
    xt = nc.dram_tensor("xt", [P, NT2, D // P, LT2], MMDT, kind="ExternalInput").ap()
    xn = nc.dram_tensor("xn", [P, L // P, D], BF16, kind="ExternalInput").ap()
    ef = nc.dram_tensor("ef", [P, L // P, 2 * K], BF16, kind="ExternalInput").ap()
    wq = nc.dram_tensor("wq", [D, DH], MMDT, kind="ExternalInput").ap()
    wk = nc.dram_tensor("wk", [D, DH], MMDT, kind="ExternalInput").ap()
    wv = nc.dram_tensor("wv", [D, DH], MMDT, kind="ExternalInput").ap()
    wo = nc.dram_tensor("wo", [DH, D], BF16, kind="ExternalInput").ap()
    bqs = nc.dram_tensor("bqs", [P, DH // P], F32, kind="ExternalInput").ap()
    # r1 = [bk (512) | SE (256) | SF (256) | bv (512) | -12 | pad] rank-1
    # operands + small constants
    r1 = nc.dram_tensor("r1", [1, 1544], MMDT, kind="ExternalInput").ap()
    idbr = nc.dram_tensor("identb", [P, P], BF16, kind="ExternalInput").ap()
    ydr = nc.dram_tensor("y", [L, D], BF16, kind="ExternalOutput").ap()

    with tile.TileContext(nc) as tc:
        with (
            tc.tile_pool(name="const", bufs=1) as constp,
            tc.tile_pool(name="persist", bufs=1) as persist,
            tc.tile_pool(name="w2", bufs=1) as w2p,
        ):
            # persistent SBUF tensors
            kET_sb = persist.tile([P, 4, K], MMDT)      # [pair-row, pair, k]
            # vF augmented with 64 ones-columns per head: pv then yields
            # out rows 0:64 and Z (softmax denom) replicated in rows 64:128
            vF_aug = persist.tile([P, 2, NHL, P], BF16)
            # exp bias: -(global score max), computed once at tile 0 head 0
            negM = persist.tile([P, 1], F32, name="negM")

            # phase-2 weights (DMAs emitted later, during phase-1 slices)
            wq_sb = w2p.tile([P, D // P, DH], MMDT, tag="wq")
            wo_sb = w2p.tile([P, DH // P, D], BF16, tag="wo")

            # phase-2 xT tile pool lives at outer scope so tile 0 can be
            # prefetched during phase 1; 3 buffers = 2 tiles of lookahead
            xt2p_cm = tc.tile_pool(name="xt2", bufs=3)
            xt2p = xt2p_cm.__enter__()

            # ------- phase 1: z = xn^T [E|F]  ->  kET, vF -------
            with tc.tile_pool(name="w1", bufs=1) as w1:
                wk_sb = w1.tile([P, D // P, DH], MMDT, tag="wk")
                wv_sb = w1.tile([P, D // P, DH], MMDT, tag="wv")
                z_sb = w1.tile([P, D // P, 2 * K], MMDT, tag="z")

                with (
                    tc.tile_pool(name="xnp", bufs=4) as xnp,
                    tc.tile_pool(name="efp", bufs=4) as efp,
                    tc.tile_pool(name="ps_z", bufs=1, space="PSUM") as ps_z,
                ):

                    def fetch_slice(ls, fine=False):
                        # half-slice DMAs: 4KB(xn)/2KB(ef) contiguous per
                        # partition line, spread over 4 queues. fine=True
                        # fetches per-chunk so the first matmul starts early.
                        xn_sl = xnp.tile([P, LS1 // P, D], BF16, tag="xn")
                        ef_sl = efp.tile([P, LS1 // P, 2 * K], BF16, tag="ef")
                        a0 = ls * (LS1 // P)
                        hh = 1 if fine else LS1 // P // 2
                        for a in range(0, LS1 // P, hh):
                            nc.sync.dma_start(
                                xn_sl[:, a : a + hh, :],
                                xn[:, a0 + a : a0 + a + hh, :],
                            )
                            nc.sync.dma_start(
                                ef_sl[:, a : a + hh, :],
                                ef[:, a0 + a : a0 + a + hh, :],
                            )
                        return xn_sl, ef_sl

                    sl_buf = {0: fetch_slice(0, fine=True)}
                    # small constants ride behind the first slice
                    identb_t = constp.tile([P, P], BF16, name="identb_t")
                    nc.sync.dma_start(identb_t[:], idbr)
                    identb = identb_t[:]
                    bqs_sb = constp.tile([P, DH // P], F32)
                    nc.sync.dma_start(bqs_sb[:], bqs)
                    r1_sb = constp.tile([1, 1544], MMDT)
                    nc.sync.dma_start(r1_sb[:], r1)
                    sl_buf[1] = fetch_slice(1)
                    sl_buf[2] = fetch_slice(2)

                    zacc = [
                        ps_z.tile([P, 2 * K], F32, tag=f"z{i}", name=f"zacc{i}")
                        for i in range(D // P)
                    ]

                    # ones columns of vF_aug (no data deps; GPSIMD is idle)
                    for kc in range(2):
                        nc.gpsimd.memset(vF_aug[:, kc, :, HD:P], 1.0)

                    for ls in range(NS1):
                        xn_sl, ef_sl = sl_buf.pop(ls)
                        if ls + 3 < NS1:
                            sl_buf[ls + 3] = fetch_slice(ls + 3)
                        # weight loads go behind the full slice stream (the
                        # DMA engines are a serial shared resource); they are
                        # needed only at/after the z-loop tail
                        if ls == 5:
                            nc.sync.dma_start(
                                wk_sb[:], wk.rearrange("(c p) n -> p c n", p=P)
                            )
                            nc.sync.dma_start(
                                wv_sb[:], wv.rearrange("(c p) n -> p c n", p=P)
                            )
                        elif ls == 6:
                            nc.sync.dma_start(
                                wq_sb[:], wq.rearrange("(c p) n -> p c n", p=P)
                            )
                            # prefetch phase-2 tile 0 xT
                            xt2_pre = xt2p.tile([P, D // P, LT2], MMDT, tag="xt2")
                            nc.sync.dma_start(xt2_pre[:], xt[:, 0, :, :])
                        elif ls == 7:
                            nc.sync.dma_start(
                                wo_sb[:], wo.rearrange("(c p) n -> p c n", p=P)
                            )
                        first = ls == 0
                        last = ls == NS1 - 1
                        for a in range(LS1 // P):
                            for dc in range(D // P):
                                nc.tensor.matmul(
                                    zacc[dc][:],
                                    xn_sl[:, a, dc * P : (dc + 1) * P],
                                    ef_sl[:, a, :],
                                    start=(first and a == 0),
                                    stop=(last and a == LS1 // P - 1),
                                )
                    # drain z to SBUF, alternating engines; dc0's group stops
                    # 7 matmuls before dc7's so drains overlap the PE tail
                    for dc in range(D // P):
                        if dc % 2 == 0:
                            nc.scalar.copy(z_sb[:, dc, :], zacc[dc][:])
                        else:
                            nc.vector.tensor_copy(z_sb[:, dc, :], zacc[dc][:])

                with tc.tile_pool(name="ps_kv", bufs=1, space="PSUM") as ps_kv:
                    # kET[mc] = sum_dc Wk[dc, mc]^T z[dc] + bk[mc] x SE
                    kps = ps_kv.tile([P, 4, K], F32, tag="kps", name="kps")
                    for mc in range(4):
                        for dc in range(D // P):
                            nc.tensor.matmul(
                                kps[:, mc, :],
                                wk_sb[:, dc, mc * P : (mc + 1) * P],
                                z_sb[:, dc, 0:K],
                                start=(dc == 0),
                                stop=False,
                            )
                        nc.tensor.matmul(
                            kps[:, mc, :],
                            r1_sb[0:1, mc * P : (mc + 1) * P],
                            r1_sb[0:1, 512 : 512 + K],
                            start=False,
                            stop=True,
                        )
                    # vF[mc] = sum_dc zF[dc, mc]^T Wv[dc] + SF[mc] x bv
                    vps = ps_kv.tile([P, 2, DH], F32, tag="vps", name="vps")
                    for mc in range(2):
                        for dc in range(D // P):
                            nc.tensor.matmul(
                                vps[:, mc, :],
                                z_sb[:, dc, K + mc * P : K + (mc + 1) * P],
                                wv_sb[:, dc, :],
                                start=(dc == 0),
                                stop=False,
                            )
                        nc.tensor.matmul(
                            vps[:, mc, :],
                            r1_sb[0:1, 768 + mc * P : 768 + (mc + 1) * P],
                            r1_sb[0:1, 1024:1536],
                            start=False,
                            stop=True,
                        )
                    nc.scalar.copy(kET_sb[:, 0:2, :], kps[:, 0:2, :])
                    nc.scalar.copy(kET_sb[:, 2:4, :], kps[:, 2:4, :])
                    for kc in range(2):
                        nc.vector.tensor_copy(
                            vF_aug[:, kc, :, 0:HD],
                            vps[:, kc, :].rearrange("p (h d) -> p h d", d=HD),
                        )

            # ---------------- phase 2 ----------------
            with (
                tc.tile_pool(name="qt", bufs=2) as qtp,
                tc.tile_pool(name="sTp", bufs=6) as sTp,
                tc.tile_pool(name="otp", bufs=4) as otp,
                tc.tile_pool(name="yp", bufs=8) as yp,
                tc.tile_pool(name="stat", bufs=4) as stat,
                tc.tile_pool(name="rcpp", bufs=4) as rcpp,
                tc.tile_pool(name="ps_q", bufs=2, space="PSUM") as ps_q,
                tc.tile_pool(name="ps_sc", bufs=2, space="PSUM") as ps_sc,
                tc.tile_pool(name="ps_out", bufs=2, space="PSUM") as ps_out,
            ):

                def emit_q_rc(xt_tile, qT_tile, rc):
                    pq = ps_q.tile([P, LT2], F32, tag="pq", name="pq")
                    for dc in range(D // P):
                        nc.tensor.matmul(
                            pq[:],
                            wq_sb[:, dc, rc * P : (rc + 1) * P],
                            xt_tile[:, dc, :],
                            start=(dc == 0),
                            stop=(dc == D // P - 1),
                        )
                    # qT = SCALE*(x@Wq) + SCALE*bq  (bqs pre-scaled on host)
                    nc.scalar.activation(
                        qT_tile[:, rc, :],
                        pq[:],
                        mybir.ActivationFunctionType.Identity,
                        bias=bqs_sb[:, rc : rc + 1],
                        scale=SCALE,
                    )

                def emit_scores_T(h, qT_t):
                    # sT-layout scores: psT[k, l] = kET_h^T q_h, 2 k-chunks
                    po = (h % 2) * HD
                    pair = h // 2
                    psT = ps_sc.tile([P, 2, LT2], F32, tag="sc", name="psT")
                    for kc in range(2):
                        nc.tensor.matmul(
                            psT[:, kc, :],
                            kET_sb[po : po + HD, pair, kc * P : (kc + 1) * P],
                            qT_t[po : po + HD, pair, :],
                            start=True,
                            stop=True,
                        )
                    return psT

                def emit_maxchain(qT_t):
                    # Softmax is shift-invariant, so one global shift M works
                    # as long as every row's max stays within exp's dynamic
                    # range of it (~±75 in f32/bf16). Row maxes span ~120
                    # here, so center M between the global max and the
                    # smallest row max (+12 tilt for the pv-accum overflow
                    # side), estimated once from tile-0/head-0 scores
                    # computed in natural [l, k] layout (row stats are then
                    # free-dim reductions; the k-partition spread is bridged
                    # with one tiny PE transpose).
                    pnat = ps_sc.tile([P, 4, K], F32, tag="sc", name="pnat")
                    for j in range(4):
                        nc.tensor.matmul(
                            pnat[:, j, :],
                            qT_t[0:HD, 0, j * P : (j + 1) * P],
                            kET_sb[0:HD, 0, :],
                            start=True,
                            stop=True,
                        )
                    rmx = stat.tile([P, 4], F32, tag="rmx")
                    nc.vector.reduce_max(rmx[:], pnat[:], axis=mybir.AxisListType.X)
                    nrmx = stat.tile([P, 4], F32, tag="nrmx")
                    nc.vector.tensor_scalar_mul(nrmx[:], rmx[:], -1.0)
                    stf = stat.tile([P, 2], F32, tag="stf")
                    nc.vector.reduce_max(
                        stf[:, 0:1], rmx[:], axis=mybir.AxisListType.X
                    )
                    nc.vector.reduce_max(
                        stf[:, 1:2], nrmx[:], axis=mybir.AxisListType.X
                    )
                    stb = stat.tile([P, 2], BF16, tag="stb")
                    nc.vector.tensor_copy(stb[:], stf[:])
                    # transpose each stat column separately so every
                    # downstream access stays on partition 0
                    ptm = ps_out.tile([1, 2, P], BF16, tag="out", name="ptm")
                    nc.tensor.transpose(ptm[0:1, 0, :], stb[:, 0:1], identb)
                    nc.tensor.transpose(ptm[0:1, 1, :], stb[:, 1:2], identb)
                    gmax1 = stat.tile([1, 1], F32, tag="gmax1")
                    nc.vector.reduce_max(
                        gmax1[:], ptm[0:1, 0, :], axis=mybir.AxisListType.X
                    )
                    ngmin1 = stat.tile([1, 1], F32, tag="ngmin1")
                    nc.vector.reduce_max(
                        ngmin1[:], ptm[0:1, 1, :], axis=mybir.AxisListType.X
                    )
                    gsum = stat.tile([1, 1], F32, tag="gsum")
                    nc.vector.tensor_sub(gsum[:], gmax1[:], ngmin1[:])
                    negm1 = stat.tile([1, 1], F32, tag="negm1")
                    nc.scalar.activation(
                        negm1[:],
                        gsum[:],
                        mybir.ActivationFunctionType.Identity,
                        bias=r1_sb[0:1, 1536:1537],
                        scale=-0.5,
                    )
                    nc.gpsimd.partition_broadcast(negM[:], negm1[:])

                def emit_exp(h, psT):
                    # split per k-chunk so pv's first matmul only waits on
                    # half the exp
                    sT = sTp.tile([P, 2, LT2], BF16, tag="sT")
                    for kc in range(2):
                        nc.scalar.activation(
                            sT[:, kc, :],
                            psT[:, kc, :],
                            mybir.ActivationFunctionType.Exp,
                            bias=negM[:, 0:1],
                            scale=1.0,
                        )
                    return sT

                def emit_pv(h, sT, outT_t):
                    po = (h % 2) * HD
                    pair = h // 2
                    pp = ps_out.tile([P, LT2], F32, tag="out", name="pp")
                    for kc in range(2):
                        nc.tensor.matmul(
                            pp[:],
                            vF_aug[:, kc, h, :],
                            sT[:, kc, :],
                            start=(kc == 0),
                            stop=(kc == 1),
                        )
                    # rows 64:128 all hold Z (ones-columns); stage Z in SBUF
                    # (ACT; GPSIMD cannot read PSUM), approx-reciprocal +
                    # multiply on DVE
                    zr = rcpp.tile([HD, LT2], F32, tag="rcp")
                    nc.scalar.copy(zr[:], pp[HD:P, :])
                    rcpb = rcpp.tile([HD, LT2], F32, tag="rcpb")
                    nc.vector.reciprocal_approx_fast(rcpb[:], zr[:])
                    nc.vector.tensor_mul(
                        outT_t[po : po + HD, pair, :], pp[0:HD, :], rcpb[:]
                    )

                def emit_y(j, outT_t, l0):
                    y_sb = yp.tile([P, D], BF16, tag="y")
                    for nh in range(2):
                        py = ps_out.tile([P, DH], F32, tag="out", name="py")
                        for c in range(4):
                            nc.tensor.matmul(
                                py[:],
                                outT_t[:, c, j * P : (j + 1) * P],
                                wo_sb[:, c, nh * DH : (nh + 1) * DH],
                                start=(c == 0),
                                stop=(c == 3),
                            )
                        if nh == 0:
                            nc.vector.tensor_copy(
                                y_sb[:, nh * DH : (nh + 1) * DH], py[:]
                            )
                        else:
                            nc.scalar.copy(y_sb[:, nh * DH : (nh + 1) * DH], py[:])
                        # ship each half as soon as its drain lands
                        nc.sync.dma_start(
                            ydr[l0 + j * P : l0 + (j + 1) * P,
                                nh * DH : (nh + 1) * DH],
                            y_sb[:, nh * DH : (nh + 1) * DH],
                        )

                # prologue: q(0) — xT tile 0 was prefetched in phase 1.
                # The softmax-shift chain runs right after q rc0 so negM is
                # ready long before tile 0's first exp.
                # prologue: q(0) — xT tile 0 was prefetched in phase 1.
                # The softmax-shift chain runs right after q rc0 so negM is
                # ready long before tile 0's first exp.
                qT_next = qtp.tile([P, 4, LT2], MMDT, tag="qt")
                emit_q_rc(xt2_pre, qT_next, 0)
                emit_maxchain(qT_next)
                for rc in range(1, 4):
                    emit_q_rc(xt2_pre, qT_next, rc)
                qT_cur = qT_next

                def fetch_xt2(t):
                    xt2_t = xt2p.tile([P, D // P, LT2], MMDT, tag="xt2")
                    nc.sync.dma_start(xt2_t[:], xt[:, t, :, :])
                    return xt2_t

                xt2_tiles = {1: fetch_xt2(1)}
                outT_prev = None
                l0_prev = 0
                for lt in range(NT2):
                    l0 = lt * LT2
                    if lt + 2 < NT2:
                        xt2_tiles[lt + 2] = fetch_xt2(lt + 2)
                    xt2_next = xt2_tiles.pop(lt + 1, None)
                    have_q = lt + 1 < NT2
                    have_y = outT_prev is not None
                    if have_q:
                        qT_next = qtp.tile([P, 4, LT2], MMDT, tag="qt")

                    # schedule: scores/exp/pv per head trickle through 2
                    # score PSUM buffers; q(lt+1) and y(lt-1) fill 8 PE gap
                    # slots, interleaved when both are present
                    fillers = []
                    if have_y and have_q:
                        fillers = [
                            lambda i=i: (
                                emit_y(i // 2, outT_prev, l0_prev)
                                if i % 2 == 0
                                else emit_q_rc(xt2_next, qT_next, i // 2)
                            )
                            for i in range(8)
                        ]
                    elif have_q:
                        fillers = [None] * 8
                        for i, rc in zip((1, 3, 5, 6), range(4)):
                            fillers[i] = (
                                lambda rc=rc: emit_q_rc(xt2_next, qT_next, rc)
                            )

                    elif have_y:
                        fillers = [None] * 8
                        for i, j in zip((0, 2, 4, 6), range(4)):
                            fillers[i] = (
                                lambda j=j: emit_y(j, outT_prev, l0_prev)
                            )

                    def fill(i):
                        if i < len(fillers) and fillers[i] is not None:
                            fillers[i]()

                    psTs = []
                    sTs = []
                    psTs.append(emit_scores_T(0, qT_cur))
                    psTs.append(emit_scores_T(1, qT_cur))
                    sTs.append(emit_exp(0, psTs[0]))
                    outT_t = otp.tile([P, 4, LT2], BF16, tag="outT")
                    fill(0)
                    psTs.append(emit_scores_T(2, qT_cur))
                    emit_pv(0, sTs[0], outT_t)
                    sTs.append(emit_exp(1, psTs[1]))
                    fill(1)
                    psTs.append(emit_scores_T(3, qT_cur))
                    emit_pv(1, sTs[1], outT_t)
                    sTs.append(emit_exp(2, psTs[2]))
                    fill(2)
                    psTs.append(emit_scores_T(4, qT_cur))
                    emit_pv(2, sTs[2], outT_t)
                    sTs.append(emit_exp(3, psTs[3]))
                    fill(3)
                    psTs.append(emit_scores_T(5, qT_cur))
                    emit_pv(3, sTs[3], outT_t)
                    sTs.append(emit_exp(4, psTs[4]))
                    fill(4)
                    psTs.append(emit_scores_T(6, qT_cur))
                    emit_pv(4, sTs[4], outT_t)
                    sTs.append(emit_exp(5, psTs[5]))
                    fill(5)
                    psTs.append(emit_scores_T(7, qT_cur))
                    emit_pv(5, sTs[5], outT_t)
                    sTs.append(emit_exp(6, psTs[6]))
                    sTs.append(emit_exp(7, psTs[7]))
                    fill(6)
                    emit_pv(6, sTs[6], outT_t)
                    fill(7)
                    emit_pv(7, sTs[7], outT_t)
                    outT_prev = outT_t
                    l0_prev = l0
                    if have_q:
                        qT_cur = qT_next
                # epilogue: y(7) — ps_q/ps_sc are idle here; emit all 8
                # matmul groups first (rotating over their 4 buffers), then
                # all drains, so the PE never waits on the drain cadence
                epi_pys = {}
                for j in range(4):
                    for nh in range(2):
                        if (2 * j + nh) % 2:
                            py = ps_q.tile(
                                [P, DH], F32, tag="pq", name="py_epi"
                            )[:]
                        else:
                            py = ps_sc.tile(
                                [P, 2, LT2], F32, tag="sc", name="py_epi"
                            )[:, 0, :]
                        epi_pys[(j, nh)] = py
                        for c in range(4):
                            nc.tensor.matmul(
                                py,
                                outT_prev[:, c, j * P : (j + 1) * P],
                                wo_sb[:, c, nh * DH : (nh + 1) * DH],
                                start=(c == 0),
                                stop=(c == 3),
                            )
                for j in range(4):
                    y_sb = yp.tile([P, D], BF16, tag="y")
                    for nh in range(2):
                        py = epi_pys[(j, nh)]
                        if nh == 0:
                            nc.vector.tensor_copy(
                                y_sb[:, nh * DH : (nh + 1) * DH], py
                            )
                        else:
                            nc.scalar.copy(y_sb[:, nh * DH : (nh + 1) * DH], py)
                        nc.sync.dma_start(
                            ydr[l0_prev + j * P : l0_prev + (j + 1) * P,
                                nh * DH : (nh + 1) * DH],
                            y_sb[:, nh * DH : (nh + 1) * DH],
                        )
            xt2p_cm.__exit__(None, None, None)
    nc.compile()
    return nc


def _get_program():
    if "nc" not in _CACHE:
        _CACHE["nc"] = build_program()
    return _CACHE["nc"]


def _shard_inputs(inputs):
    x = np.asarray(inputs["x"], np.float32)
    Wq = np.asarray(inputs["Wq"], np.float32)
    bq = np.asarray(inputs["bq"], np.float32)
    Wk = np.asarray(inputs["Wk"], np.float32)
    bk = np.asarray(inputs["bk"], np.float32)
    Wv = np.asarray(inputs["Wv"], np.float32)
    bv = np.asarray(inputs["bv"], np.float32)
    E = np.ascontiguousarray(np.asarray(inputs["E"], np.float32))
    F = np.ascontiguousarray(np.asarray(inputs["F"], np.float32))
    Wo = np.asarray(inputs["Wo"], np.float32)
    # swizzled layouts: partition-major so DMAs read long contiguous runs
    # ef[p, i, :] = [E|F][i*128 + p, :]
    ef = np.ascontiguousarray(
        np.hstack([E, F]).astype(ml_dtypes.bfloat16)
        .reshape(L // P, P, 2 * K).transpose(1, 0, 2)
    )
    SE, SF = E.sum(0), F.sum(0)
    identb = np.eye(P, dtype=ml_dtypes.bfloat16)
    # xn[p, i, :] = x[b][i*128 + p, :]
    xns = [
        np.ascontiguousarray(
            x[b].astype(ml_dtypes.bfloat16)
            .reshape(L // P, P, D).transpose(1, 0, 2)
        )
        for b in range(4)
    ]
    # xt[p, t, c, :] = x[b].T[c*128 + p, t*512:(t+1)*512]
    xts = [
        np.ascontiguousarray(
            x[b].T.reshape(D // P, P, L // 512, 512).transpose(1, 2, 0, 3)
        )
        for b in range(4)
    ]
    in_maps = []
    for c in range(8):
        b, hh = c // 2, c % 2
        sl = slice(hh * DH, (hh + 1) * DH)
        r1 = np.concatenate(
            [bk[sl], SE, SF, bv[sl], [-12.0], np.zeros(7, np.float32)]
        ).reshape(1, 1544)
        in_maps.append(
            {
                "xt": xts[b],
                "xn": xns[b],
                "ef": ef,
                "wq": np.ascontiguousarray(Wq[:, sl]),
                "wk": np.ascontiguousarray(Wk[:, sl]),
                "wv": np.ascontiguousarray(Wv[:, sl]),
                "wo": np.ascontiguousarray(Wo[sl, :].astype(ml_dtypes.bfloat16)),
                "bqs": np.ascontiguousarray(
                    (bq[sl] * SCALE).reshape(4, P).T.astype(np.float32)
                ),
                "r1": np.ascontiguousarray(r1.astype(np.float32)),
                "identb": identb,
            }
        )
    return in_maps


def _ensure_profile_hook():
    """The container's `antenv` stub lacks `axon_hooks`; synthesize it so
    run_bass_kernel_spmd(trace=True) can reach the NTFF capture ABI in
    libaxon_pjrt.so (see trn_agent_boot.trn_boot)."""
    import types
    import antenv

    if hasattr(antenv, "axon_hooks"):
        return
    mod = types.ModuleType("antenv.axon_hooks")
    _state = {"hook": None}
    mod.set_axon_ntff_profile_hook = lambda h: _state.__setitem__("hook", h)
    mod.get_axon_ntff_profile_hook = lambda: _state["hook"]
    sys.modules["antenv.axon_hooks"] = mod
    antenv.axon_hooks = mod
    try:
        from trn_agent_boot.trn_boot import _ntff_profile_via_ctypes

        mod.set_axon_ntff_profile_hook(
            _ntff_profile_via_ctypes("/opt/axon/libaxon_pjrt.so")
        )
    except Exception as e:
        print(f"profile hook setup failed: {e}", file=sys.stderr)


def run(inputs, trace=False, **kw):
    if trace:
        _ensure_profile_hook()
    nc = _get_program()
    in_maps = _shard_inputs(inputs)
    res = bass_utils.run_bass_kernel_spmd(
        nc, in_maps, core_ids=list(range(8)), trace=trace, **kw
    )
    bo = np.asarray(inputs["bo"], np.float32)
    x = np.asarray(inputs["x"], np.float32)
    Bc = x.shape[0]
    y = np.empty((Bc, L, D), np.float32)
    for b in range(Bc):
        y[b] = (
            np.asarray(res.results[2 * b]["y"], np.float32)
            + np.asarray(res.results[2 * b + 1]["y"], np.float32)
            + bo
        )
    return y, res


def kernel(**inputs):
    n_heads = int(inputs.get("n_heads", H))
    assert n_heads == H, f"kernel hardcoded for {H} heads, got {n_heads}"
    y, _ = run(inputs, trace=False)
    return y

